# revision 17
# baseline (speedup 1.0000x reference)
"""Trainium2 Bass kernel for nn_DecoderBlock (dense transformer decoder block).

Sharding: data-parallel over batch (8 batch elements -> 8 NeuronCores), no
collectives. Each core computes one full decoder block on [S=1024, D=1024].

Per-core strategy:
  - activations kept feature-major ([D, S]) so every Linear is
    out = W_tile.T @ actT with W streamed from HBM unchanged ([in, out])
  - LN1 in natural layout (free-dim bn_stats) fused into the input transpose;
    LN2/LN3 feature-major with partition-dim stats via ones-matmuls and K=1
    broadcast matmuls
  - softmax without max-subtraction (scores are small); causal mask via
    gpsimd affine_select on the exp'd tiles; V carries an appended ones
    column so softmax denominators fall out of the attn@v matmul
  - all matmuls in float32r (fp32 bits, PE rounds; ~1.5e-4 rel err/matmul,
    4x faster than plain fp32)
"""
import sys

sys.path.insert(0, '/opt/trn_rl_repo')

import contextlib

import numpy as np

import concourse.bacc as bacc
import concourse.mybir as mybir
import concourse.tile as tile
from concourse.bass_utils import run_bass_kernel_spmd
from concourse.masks import make_identity

f32 = mybir.dt.float32
f32r = mybir.dt.float32r
AF = mybir.ActivationFunctionType
ALU = mybir.AluOpType

B, S, D, H, HD, FF = 8, 1024, 1024, 16, 64, 4096
ST = S // 128   # 8
DT = D // 128   # 8
FT = FF // 128  # 32
EPS = 1e-5
ISQ = 1.0 / 8.0  # 1/sqrt(HD)

W_NAMES = ['sa_wq', 'sa_wk', 'sa_wv', 'sa_wo', 'ca_wq', 'ca_wk', 'ca_wv', 'ca_wo']
B_NAMES = ['sa_bq', 'sa_bk', 'sa_bv', 'sa_bo', 'ca_bq', 'ca_bk', 'ca_bv', 'ca_bo']
LN_NAMES = ['ln1_g', 'ln1_b', 'ln2_g', 'ln2_b', 'ln3_g', 'ln3_b']


def _build(iters=1):
    nc = bacc.Bacc("TRN2", target_bir_lowering=False, debug=False, num_devices=8)

    dec_d = nc.dram_tensor("decoder", [S, D], f32, kind="ExternalInput").ap()
    enc_d = nc.dram_tensor("encoder", [S, D], f32, kind="ExternalInput").ap()
    wd = {n: nc.dram_tensor(n, [D, D], f32, kind="ExternalInput").ap() for n in W_NAMES}
    bd = {n: nc.dram_tensor(n, [1, D] if n.endswith('bv') else [D], f32,
                            kind="ExternalInput").ap() for n in B_NAMES}
    lnd = {n: nc.dram_tensor(n, [D], f32, kind="ExternalInput").ap() for n in LN_NAMES}
    w1_d = nc.dram_tensor("ffn_w1", [D, FF], f32, kind="ExternalInput").ap()
    b1_d = nc.dram_tensor("ffn_b1", [FF], f32, kind="ExternalInput").ap()
    w2_d = nc.dram_tensor("ffn_w2", [FF, D], f32, kind="ExternalInput").ap()
    b2_d = nc.dram_tensor("ffn_b2", [D], f32, kind="ExternalInput").ap()
    out_d = nc.dram_tensor("out", [S, D], f32, kind="ExternalOutput").ap()

    with tile.TileContext(nc) as tc, \
            nc.allow_low_precision(reason="fp32r matmul pipeline by design"):
        _body(nc, tc, dec_d, enc_d, wd, bd, lnd, w1_d, b1_d, w2_d, b2_d, out_d, iters)
    nc.compile()
    return nc


def _body(nc, tc, dec_d, enc_d, wd, bd, lnd, w1_d, b1_d, w2_d, b2_d, out_d, iters):
    ctx = contextlib.ExitStack()
    with ctx:
        persist = ctx.enter_context(tc.tile_pool(name="persist", bufs=1))
        big = ctx.enter_context(tc.tile_pool(name="big", bufs=1))
        grp = ctx.enter_context(tc.tile_pool(name="grp", bufs=1))
        att = ctx.enter_context(tc.tile_pool(name="att", bufs=3))
        wp = ctx.enter_context(tc.tile_pool(name="wp", bufs=2))
        w2p = ctx.enter_context(tc.tile_pool(name="w2p", bufs=2))
        sm = ctx.enter_context(tc.tile_pool(name="sm", bufs=2))
        ps_a = ctx.enter_context(tc.tile_pool(name="ps_a", bufs=2, space="PSUM"))
        ps_r = ctx.enter_context(tc.tile_pool(name="ps_r", bufs=4, space="PSUM"))

        # ---- persistent constants ----
        ones_f = persist.tile([128, 4], f32, tag="ones_f")
        nc.vector.memset(ones_f, 1.0)
        ones_col = persist.tile([128, 1], f32r, tag="ones_col")
        nc.vector.tensor_copy(ones_col, ones_f[:, 0:1])
        onesr_f = persist.tile([1, 128], f32, tag="onesr_f")
        nc.vector.memset(onesr_f, 1.0)
        ones_row = persist.tile([1, 128], f32r, tag="ones_row")
        nc.vector.tensor_copy(ones_row, onesr_f)
        ident = persist.tile([128, 128], f32, tag="ident")
        make_identity(nc, ident)
        ident_r = persist.tile([128, 128], f32r, tag="ident_r")
        nc.vector.tensor_copy(ident_r, ident)
        eps1 = persist.tile([1, 1], f32, tag="eps1")
        nc.vector.memset(eps1, EPS)
        eps128 = persist.tile([128, 1], f32, tag="eps128")
        nc.vector.memset(eps128, EPS)

        bias_t = {}
        for n in ['sa_bq', 'sa_bk', 'sa_bo', 'ca_bq', 'ca_bk', 'ca_bo']:
            bias_t[n] = persist.tile([128, DT], f32, tag=n, name=n)
            nc.sync.dma_start(bias_t[n], bd[n].rearrange("(t p) -> p t", p=128))
        for n in LN_NAMES:
            bias_t[n] = persist.tile([128, DT], f32, tag=n, name=n)
            nc.sync.dma_start(bias_t[n], lnd[n].rearrange("(t p) -> p t", p=128))
        bias_t['ffn_b1'] = persist.tile([128, FT], f32, tag="ffn_b1", name="ffn_b1")
        nc.sync.dma_start(bias_t['ffn_b1'], b1_d.rearrange("(t p) -> p t", p=128))
        bias_t['ffn_b2'] = persist.tile([128, DT], f32, tag="ffn_b2", name="ffn_b2")
        nc.sync.dma_start(bias_t['ffn_b2'], b2_d.rearrange("(t p) -> p t", p=128))
        bvg_t = persist.tile([1, 256], f32r, tag="bvg")

        # stats scratch rows (single-buffered, reused per LN/softmax call)
        a_row = persist.tile([1, S], f32r, tag="a_row")
        c_row = persist.tile([1, S], f32r, tag="c_row")
        rowA = persist.tile([1, 512], f32, tag="rowA")
        rowB = persist.tile([1, 512], f32, tag="rowB")
        rowC = persist.tile([1, 512], f32, tag="rowC")
        rec = persist.tile([1, 512], f32r, tag="rec")

        # ---- big activation buffers [128, 8, 1024] (4 MB each) ----
        # natrep: dec-natural/LN1 -> SA repT -> enc-natural -> CA repT -> h half1
        # bufA:   xT -> encT -> h half2
        # bufB:   x2T/yT -> outT
        # bufC:   y2T/zT
        natrep = big.tile([128, ST, D], f32, tag="natrep")
        bufA = big.tile([128, DT, S], f32, tag="bufA")
        bufB = big.tile([128, DT, S], f32, tag="bufB")
        bufC = big.tile([128, DT, S], f32, tag="bufC")
        hbufs = [natrep, bufA]

        def mm(out_ap, lhsT_ap, rhs_ap, start, stop):
            nc.tensor.matmul(out_ap, lhsT_ap, rhs_ap, start=start, stop=stop,
                             skip_group_check=True)

        def load_w_cols(w_dram, m, kt=DT, tag="wcol"):
            t = wp.tile([128, kt, 128], f32r, tag=tag)
            src_ap = (w_dram[:, m * 128:(m + 1) * 128]
                      .rearrange("(k p) q -> p k q", p=128).bitcast(f32r))
            kh = kt // 2
            nc.sync.dma_start(t[:, 0:kh, :], src_ap[:, 0:kh, :])
            nc.sync.dma_start(t[:, kh:kt, :], src_ap[:, kh:kt, :])
            return t

        def proj_T(w_dram, bias, src_T, dst_T, residual=None):
            for m in range(DT):
                wt = load_w_cols(w_dram, m)
                ps = ps_a.tile([128, S], f32, tag="a")
                for k in range(DT):
                    for c in range(2):
                        cs = slice(c * 512, (c + 1) * 512)
                        mm(ps[:, cs], wt[:, k, :], src_T[:, k, cs].bitcast(f32r),
                           k == 0, k == DT - 1)
                if residual is None:
                    nc.vector.tensor_scalar(
                        dst_T[:, m, :].bitcast(f32r), ps, bias[:, m:m + 1], None,
                        ALU.add)
                else:
                    nc.vector.scalar_tensor_tensor(
                        dst_T[:, m, :].bitcast(f32r), ps, bias[:, m:m + 1],
                        residual[:, m, :], ALU.add, ALU.add)

        def transpose_to(src, dst_T, gb=None, src_r=False):
            # block-transpose [*,a,b]-major -> [*,b,a]-major, 128x128 PE blocks
            for i in range(ST):
                for j in range(DT):
                    tp = ps_r.tile([128, 512], f32, tag="r")
                    in_ap = src[:, i, j * 128:(j + 1) * 128]
                    if src_r:
                        nc.tensor.transpose(tp[:, 0:128].bitcast(f32r), in_ap.bitcast(f32r), ident_r)
                    else:
                        nc.tensor.transpose(tp[:, 0:128], in_ap, ident)
                    dst = dst_T[:, j, i * 128:(i + 1) * 128].bitcast(f32r)
                    if gb is None:
                        nc.vector.tensor_copy(dst, tp[:, 0:128])
                    else:
                        g_ap, b_ap = gb
                        nc.vector.tensor_scalar(
                            dst, tp[:, 0:128], g_ap[:, j:j + 1], b_ap[:, j:j + 1],
                            ALU.mult, ALU.add)

        def ln_partition(T, g_ap, b_ap):
            # in-place layernorm over the feature (partition-tiled) dim of T
            sums = [ps_r.tile([1, 512], f32, tag="r", name=f"sums{_c}") for _c in range(2)]
            ssqs = [ps_r.tile([1, 512], f32, tag="r", name=f"ssqs{_c}") for _c in range(2)]
            for t in range(DT):
                for c in range(2):
                    cs = slice(c * 512, (c + 1) * 512)
                    sq = sm.tile([128, 512], f32r, tag="sq")
                    nc.vector.tensor_mul(sq, T[:, t, cs], T[:, t, cs])
                    mm(sums[c], ones_col, T[:, t, cs].bitcast(f32r), t == 0, t == DT - 1)
                    mm(ssqs[c], ones_col, sq, t == 0, t == DT - 1)
            for c in range(2):
                cs = slice(c * 512, (c + 1) * 512)
                nc.vector.tensor_scalar(rowA, sums[c], 1.0 / D, None, ALU.mult)  # mu
                nc.vector.tensor_scalar(rowB, ssqs[c], 1.0 / D, None, ALU.mult)  # E[x^2]
                nc.vector.scalar_tensor_tensor(rowC, rowA, -1.0, rowA, ALU.mult,
                                               ALU.mult)                          # -mu^2
                nc.vector.tensor_add(rowB, rowB, rowC)                            # var
                nc.scalar.activation(rowC, rowB, AF.Sqrt, bias=eps1)              # std
                nc.vector.reciprocal(rowB, rowC)                                  # rstd
                nc.vector.tensor_copy(a_row[:, cs], rowB)
                nc.vector.scalar_tensor_tensor(c_row[:, cs], rowA, -1.0, rowB,
                                               ALU.mult, ALU.mult)                # -mu*rstd
            bcA = ps_a.tile([128, S], f32, tag="a")
            bcC = ps_a.tile([128, S], f32, tag="a")
            for c in range(2):
                cs = slice(c * 512, (c + 1) * 512)
                mm(bcA[:, cs], ones_row, a_row[:, cs], True, True)
                mm(bcC[:, cs], ones_row, c_row[:, cs], True, True)
            for t in range(DT):
                for c in range(2):
                    cs = slice(c * 512, (c + 1) * 512)
                    tmp = sm.tile([128, 512], f32r, tag="sq", name="lntmp")
                    nc.vector.tensor_scalar(tmp, bcC[:, cs], g_ap[:, t:t + 1],
                                            b_ap[:, t:t + 1], ALU.mult, ALU.add)
                    nc.vector.tensor_mul(T[:, t, cs].bitcast(f32r), T[:, t, cs],
                                         bcA[:, cs])
                    nc.vector.scalar_tensor_tensor(
                        T[:, t, cs].bitcast(f32r), T[:, t, cs], g_ap[:, t:t + 1],
                        tmp, ALU.mult, ALU.add)

        def attention(src_q_T, src_kv_T, pre, causal, dst_T, residual_T, repT):
            wq, wk, wv, wo = (wd[pre + n] for n in ('wq', 'wk', 'wv', 'wo'))
            bq, bk, bo = (bias_t[pre + n] for n in ('bq', 'bk', 'bo'))
            bd_bv = bd[pre + 'bv']

            for g in range(4):
                qg = grp.tile([128, 2, S], f32, tag="qg")
                kg = grp.tile([128, 2, S], f32, tag="kg")
                vg = grp.tile([128, ST, 4, 65], f32, tag="vg")
                for mi in range(2):
                    m = 2 * g + mi
                    for wmat, bmat, dst, srcx in ((wq, bq, qg, src_q_T),
                                                  (wk, bk, kg, src_kv_T)):
                        wt = load_w_cols(wmat, m)
                        ps = ps_a.tile([128, S], f32, tag="a")
                        for k in range(DT):
                            for c in range(2):
                                cs = slice(c * 512, (c + 1) * 512)
                                mm(ps[:, cs], wt[:, k, :],
                                   srcx[:, k, cs].bitcast(f32r), k == 0, k == DT - 1)
                        nc.vector.tensor_scalar(
                            dst[:, mi, :].bitcast(f32r), ps, bmat[:, m:m + 1], None,
                            ALU.add)
                # v natural layout (+bias via K=1 bcast, +ones column)
                nc.sync.dma_start(
                    bvg_t, bd_bv[:, g * 256:(g + 1) * 256].bitcast(f32r))
                bcv = ps_r.tile([128, 512], f32, tag="r")
                mm(bcv[:, 0:256], ones_row, bvg_t, True, True)
                bcv_sb = sm.tile([128, 256], f32r, tag="sq", name="bcv_sb")
                nc.vector.tensor_copy(bcv_sb, bcv[:, 0:256])
                wvr = grp.tile([128, DT, 256], f32r, tag="wvr")
                wvr_src = (wv[:, g * 256:(g + 1) * 256]
                           .rearrange("(k p) q -> p k q", p=128).bitcast(f32r))
                nc.sync.dma_start(wvr[:, 0:4, :], wvr_src[:, 0:4, :])
                nc.sync.dma_start(wvr[:, 4:8, :], wvr_src[:, 4:8, :])
                for st in range(ST):
                    psv = ps_r.tile([128, 512], f32, tag="r")
                    for k in range(DT):
                        mm(psv[:, 0:256],
                           src_kv_T[:, k, st * 128:(st + 1) * 128].bitcast(f32r),
                           wvr[:, k, :], k == 0, k == DT - 1)
                    nc.vector.tensor_add(
                        vg[:, st, :, 0:64].bitcast(f32r),
                        psv[:, 0:256].rearrange("p (h e) -> p h e", h=4),
                        bcv_sb.rearrange("p (h e) -> p h e", h=4))
                    nc.vector.tensor_copy(
                        vg[:, st, :, 64:65].bitcast(f32r),
                        ones_f[:, 0:4].unsqueeze(2))
                for h in range(4):
                    ha = g * 4 + h
                    po = (h % 2) * 64
                    dl = h // 2
                    # contributing (skt) lists per q-chunk c
                    contrib = []
                    for c in range(2):
                        sq_lo, sq_hi = c * 512, c * 512 + 511
                        lst = [skt for skt in range(ST)
                               if not (causal and skt * 128 > sq_hi)]
                        contrib.append(lst)
                    rp = [ps_r.tile([128, 512], f32, tag="r", name=f"rp{_c}") for _c in range(2)]
                    for skt in range(ST):
                        cset = [c for c in range(2) if skt in contrib[c]]
                        if not cset:
                            continue
                        sc = ps_a.tile([128, S], f32, tag="a")
                        ats = {}
                        for c in cset:
                            cs = slice(c * 512, (c + 1) * 512)
                            mm(sc[:, cs],
                               kg[po:po + 64, dl, skt * 128:(skt + 1) * 128],
                               qg[po:po + 64, dl, cs], True, True)
                            at = att.tile([128, 512], f32r, tag="at", name="at")
                            ats[c] = at
                            nc.scalar.activation(at, sc[:, cs], AF.Exp, scale=ISQ)
                            if causal and skt * 128 + 127 > c * 512:
                                nc.gpsimd.affine_select(
                                    out=at, in_=at,
                                    compare_op=ALU.is_ge, fill=0.0,
                                    base=c * 512 - skt * 128,
                                    pattern=[[1, 512]], channel_multiplier=-1)
                        for c in cset:
                            mm(rp[c][0:65, :], vg[:, skt, h, 0:65].bitcast(f32r),
                               ats[c], skt == contrib[c][0], skt == contrib[c][-1])
                    for c in range(2):
                        cs = slice(c * 512, (c + 1) * 512)
                        nc.vector.reciprocal(rec, rp[c][64:65, :])
                        bcr = ps_r.tile([128, 512], f32, tag="r")
                        mm(bcr[0:64, :], ones_row[:, 0:64], rec, True, True)
                        bcr_sb = sm.tile([64, 512], f32r, tag="sq", name="bcr_sb")
                        nc.vector.tensor_copy(bcr_sb, bcr[0:64, :])
                        nc.vector.tensor_mul(
                            repT[(ha % 2) * 64:(ha % 2) * 64 + 64, ha // 2, cs]
                            .bitcast(f32r),
                            rp[c][0:64, :], bcr_sb)
            proj_T(wo, bo, repT, dst_T, residual=residual_T)

        # ================= block body =================
        def block_body(_i=None):
            # P0/P1: decoder natural + LN1 (free-dim stats), transpose -> xT
            dec_r = dec_d.rearrange("(t p) d -> p t d", p=128).bitcast(f32r)
            for st in range(ST):
                nc.sync.dma_start(natrep[:, st, :].bitcast(f32r), dec_r[:, st, :])
            for st in range(ST):
                stats = sm.tile([128, 2, 6], f32, tag="bnst")
                xr = natrep[:, st, :].rearrange("p (g d) -> p g d", g=2)
                for g2 in range(2):
                    nc.vector.bn_stats(stats[:, g2, :], xr[:, g2, :])
                mv = sm.tile([128, 2], f32, tag="bnmv")
                nc.vector.bn_aggr(mv, stats)
                std = sm.tile([128, 1], f32, tag="bnstd")
                nc.scalar.activation(std, mv[:, 1:2], AF.Sqrt, bias=eps128)
                rstd = sm.tile([128, 1], f32, tag="bnrstd")
                nc.vector.reciprocal(rstd, std)
                nc.vector.tensor_scalar(natrep[:, st, :].bitcast(f32r),
                                        natrep[:, st, :],
                                        mv[:, 0:1], rstd, ALU.subtract, ALU.mult)
            transpose_to(natrep, bufA, gb=(bias_t['ln1_g'], bias_t['ln1_b']))

            # P3: self-attention (causal), residual xT -> x2T (bufB)
            attention(bufA, bufA, 'sa_', True, bufB, bufA, natrep)

            # P4: LN2 in-place -> yT
            ln_partition(bufB, bias_t['ln2_g'], bias_t['ln2_b'])

            # P4.5: encoder natural -> encT (bufA)
            enc_r = enc_d.rearrange("(t p) d -> p t d", p=128).bitcast(f32r)
            for st in range(ST):
                nc.sync.dma_start(natrep[:, st, :].bitcast(f32r), enc_r[:, st, :])
            transpose_to(natrep, bufA)

            # P5: cross-attention, residual yT -> y2T (bufC)
            attention(bufB, bufA, 'ca_', False, bufC, bufB, natrep)

            # P6: LN3 in-place -> zT
            ln_partition(bufC, bias_t['ln3_g'], bias_t['ln3_b'])

            # P7: FFN in two FF halves (h tiles live in natrep/bufA)
            for hf in range(2):
                for mi in range(16):
                    ft = hf * 16 + mi
                    wt = load_w_cols(w1_d, ft)
                    ps = ps_a.tile([128, S], f32, tag="a")
                    for k in range(DT):
                        for c in range(2):
                            cs = slice(c * 512, (c + 1) * 512)
                            mm(ps[:, cs], wt[:, k, :], bufC[:, k, cs].bitcast(f32r),
                               k == 0, k == DT - 1)
                    hb = hbufs[mi // 8]
                    nc.scalar.activation(hb[:, mi % 8, :].bitcast(f32r), ps, AF.Gelu,
                                         bias=bias_t['ffn_b1'][:, ft:ft + 1],
                                         scale=1.0)
                for m in range(DT):
                    ps = ps_a.tile([128, S], f32, tag="a")
                    for half in range(2):
                        w2t = w2p.tile([128, 8, 128], f32r, tag="w2col")
                        k_lo = hf * 2048 + half * 1024
                        w2src = (w2_d[k_lo:k_lo + 1024, m * 128:(m + 1) * 128]
                                 .rearrange("(k p) q -> p k q", p=128).bitcast(f32r))
                        nc.sync.dma_start(w2t[:, 0:4, :], w2src[:, 0:4, :])
                        nc.sync.dma_start(w2t[:, 4:8, :], w2src[:, 4:8, :])
                        for k8 in range(8):
                            k2 = half * 8 + k8
                            for c in range(2):
                                cs = slice(c * 512, (c + 1) * 512)
                                mm(ps[:, cs], w2t[:, k8, :],
                                   hbufs[k2 // 8][:, k2 % 8, cs].bitcast(f32r),
                                   k2 == 0, k2 == 15)
                    if hf == 0:
                        nc.vector.scalar_tensor_tensor(
                            bufB[:, m, :].bitcast(f32r), ps, 1.0, bufC[:, m, :],
                            ALU.mult, ALU.add)
                    else:
                        nc.vector.scalar_tensor_tensor(
                            bufB[:, m, :].bitcast(f32r), ps,
                            bias_t['ffn_b2'][:, m:m + 1], bufB[:, m, :],
                            ALU.add, ALU.add)

            # P8: transpose outT (bufB) -> DRAM directly from PSUM
            for i in range(DT):
                for j in range(ST):
                    tp = ps_r.tile([128, 512], f32, tag="r")
                    nc.tensor.transpose(
                        tp[:, 0:128].bitcast(f32r),
                        bufB[:, i, j * 128:(j + 1) * 128].bitcast(f32r), ident_r)
                    ob = sm.tile([128, 128], f32, tag="outb")
                    nc.vector.tensor_copy(ob, tp[:, 0:128])
                    nc.sync.dma_start(
                        out_d[j * 128:(j + 1) * 128, i * 128:(i + 1) * 128], ob)

        if iters == 1:
            block_body()
        else:
            with tc.For_i(0, iters, 1):
                block_body()


_CACHE = {}


def _get_nc(iters=1):
    if iters not in _CACHE:
        _CACHE[iters] = _build(iters)
    return _CACHE[iters]


def _in_maps(inputs):
    shared = {}
    for n in W_NAMES + B_NAMES + LN_NAMES + ['ffn_w1', 'ffn_b1', 'ffn_w2', 'ffn_b2']:
        shared[n] = np.ascontiguousarray(np.asarray(inputs[n], dtype=np.float32))
    for n in ('sa_bv', 'ca_bv'):
        shared[n] = shared[n].reshape(1, D)
    dec = np.asarray(inputs['decoder'], dtype=np.float32)
    enc = np.asarray(inputs['encoder'], dtype=np.float32)
    maps = []
    for b in range(B):
        m = dict(shared)
        m['decoder'] = np.ascontiguousarray(dec[b])
        m['encoder'] = np.ascontiguousarray(enc[b])
        maps.append(m)
    return maps


def kernel(**inputs):
    nc = _get_nc(1)
    res = run_bass_kernel_spmd(nc, _in_maps(inputs), core_ids=list(range(B)))
    return np.stack([res.results[b]['out'] for b in range(B)], axis=0)
